# revision 34
# baseline (speedup 1.0000x reference)
"""Multi-head causal attention (B=4, C=2048, E=1024, H=16, D=64) on 8 trn2 cores.

Sharding: core i = (batch b=i//2, head-group g=i%2).  Each core computes its
batch's attention for 8 heads (512 features) and a partial output projection;
the host sums the two partials per batch (W_o split row-wise).

Single fused pipeline per core:
  - qc-major attention (hp inner) with the output projection for q-chunk qc
    emitted as PE filler inside q-chunk qc+1 -- hidden never round-trips
    through DRAM.
  - V / Q / K projection chains are *fillers*: emitted between attention
    blocks under a credit scheduler so the in-order PE queue never idles
    behind the score->exp->hidden dependency chain.
  - q/k staged in BF16: fp32(r) moving operands stream at 2 cyc/col, so the
    row-tiled score pair really costs ~width/2.4 ns in bf16 (half of f32r).
  - diagonal blocks are trimmed: only q-columns >= k are computed (scores,
    exp, hidden all shrink); the causal mask is one 128x128 triangle applied
    to the boundary strip only.
  - PSUM: st[128,1024]x2 + hid[128,1024]x1 + pp[128,1024]x1 = 8 banks.
    hid is freed via one bf16 copy to SBUF.  1/rowsum: exp(-ln) on ACT for
    most groups; a Schraudolph+2-Newton DVE chain (yielding -1/rowsum; W_o
    is negated on the host) for the groups whose normalize would land in
    the ACT-bound qc3 exp stream.
  - all inputs ride the sync sequencer's DMA queue (~356GB/s; the scalar
    one is 4-6x slower) in consumption order, descriptor-minimal host
    repacks (one dma_start per q-chunk / weight block).
  - the PE warms its HAM clock gate on a locally-memset tile from ~6.5us
    (no DMA dependency); the final q-chunk's output projection is split
    f0-f2 / f3+store so only the last f3 matmuls wait on the last
    normalize.
"""

import numpy as np

import concourse.bass as bass
import concourse.mybir as mybir
import concourse.tile as tile
from concourse.vector_clock import ScopedClock

B, C, E = 4, 2048, 1024
H, D = 16, 64
N_CORES = 8
GF = 512          # features per head-group (8 heads x 64)
HP = 4            # head-pairs per group
QC = 512          # q-chunk width
KB = 128          # k-block width
NQC = C // QC     # 4
NKB = C // KB     # 16
NE = E // 128     # 8 contraction tiles over E
F32 = mybir.dt.float32
F32R = mybir.dt.float32r
BF16 = mybir.dt.bfloat16

_CACHED_NC = None


class PatchedTC(tile.TileContext):
    """This walrus build caps sync waits per instruction (1 for CTRL, ~2 for
    compute ISA structs).  Hoist excess waits onto same-engine NOPs emitted
    just before the instruction (engine streams execute in order, so the
    semantics are identical), and split the end-of-kernel drain's waits
    across single-wait drain instructions."""

    WAIT_CAP = 1

    def _commit_instruction(self, inst, lazy_reg_writes=True):
        si = getattr(inst, "sync_info", None)
        if (
            si is not None
            and len(si.on_wait) > self.WAIT_CAP
            and getattr(inst, "engine", mybir.EngineType.Unassigned)
            != mybir.EngineType.Unassigned
        ):
            waits = list(si.on_wait)
            keep = waits[: self.WAIT_CAP]
            extra = waits[self.WAIT_CAP :]
            si.on_wait[:] = keep
            for w in extra:
                nop = mybir.InstNoOp(
                    name=f"I-nw{self.nc.next_id()}",
                    engine=inst.engine,
                    bass_nofuse=True,
                    sync_info=mybir.SyncInfo(on_wait=[w], on_update=[]),
                )
                super()._commit_instruction(nop, lazy_reg_writes=False)
        return super()._commit_instruction(inst, lazy_reg_writes)

    def _drain_and_barrier(self, tick_clock, wait_clock):
        carrier = self.nc.sync.drain()
        wait_clock.add_sem_waits(
            carrier.ins, ScopedClock({None: tick_clock.global_clock})
        )
        si = carrier.ins.sync_info
        waits = list(si.on_wait) if si is not None else []
        if len(waits) > 1:
            si.on_wait[:] = waits[:1]
            for w in waits[1:]:
                extra = self.nc.sync.drain()
                extra.ins.sync_info = mybir.SyncInfo(on_wait=[w], on_update=[])
        self.nc.all_engine_barrier()
        assert self.sems is not None
        popped = self.nc._tile_sem_poison_stack.pop()
        assert popped is self._sem_poison
        self.nc.clear_and_free_semaphores(list(self.sems.allocated().values()))
        self.nc.all_engine_barrier()


def build_nc():
    nc = bass.Bass("TRN2", target_bir_lowering=False)
    # x is repacked chunk-major on the host ([NQC, 128, NE, QC], partition-
    # line contiguous) so one q-chunk loads with a single dma_start
    # (~128 descriptors) instead of 8 strided ones.
    xT = nc.declare_dram_parameter("xT", [NQC, 128, NE, QC], BF16, isOutput=False)
    # W_q/W_k/W_v/W_o are repacked on the host into descriptor-minimal
    # layouts (contiguous per partition line) -- DGE generation is
    # ~7ns/descriptor, so the default strided layouts cost 2.5-3.6us of
    # sequencer time each
    Wq = nc.declare_dram_parameter("Wq", [HP, 128, NE * 128], BF16, isOutput=False)
    Wk = nc.declare_dram_parameter("Wk", [HP, 128, NE * 128], BF16, isOutput=False)
    Wv = nc.declare_dram_parameter("Wv", [128, NE, GF], BF16, isOutput=False)
    Wo = nc.declare_dram_parameter("Wo", [128, HP * E], BF16, isOutput=False)
    msk = nc.declare_dram_parameter("mask", [128, 128], BF16, isOutput=False)
    out = nc.declare_dram_parameter("out", [C, E], BF16, isOutput=True)

    xT_t = xT.ap()                                  # [NQC, 128, NE, QC]

    MM_NS = 216.0        # back-to-back bf16 N=512 matmul
    PAIR_NS = 228.0      # concurrent bf16 row-tiled pair, N=512

    with PatchedTC(nc) as tc:
        import contextlib

        with contextlib.ExitStack() as ctx:
            consts = ctx.enter_context(tc.tile_pool(name="consts", bufs=1))
            xpool = ctx.enter_context(tc.tile_pool(name="xpool", bufs=1))
            vpool = ctx.enter_context(tc.tile_pool(name="vpool", bufs=1))
            qkpool = ctx.enter_context(tc.tile_pool(name="qkpool", bufs=1))
            wpool = ctx.enter_context(tc.tile_pool(name="wpool", bufs=1))
            hfpool = ctx.enter_context(tc.tile_pool(name="hfpool", bufs=1))
            stpool = ctx.enter_context(tc.tile_pool(name="stp", bufs=2, space="PSUM"))
            hidpool = ctx.enter_context(tc.tile_pool(name="hidp", bufs=1, space="PSUM"))
            pppool = ctx.enter_context(tc.tile_pool(name="ppp", bufs=1, space="PSUM"))
            wtpool = ctx.enter_context(tc.tile_pool(name="wtpool", bufs=3))
            hrawpool = ctx.enter_context(tc.tile_pool(name="hrawpool", bufs=2))
            napool = ctx.enter_context(tc.tile_pool(name="napool", bufs=1))
            sopool = ctx.enter_context(tc.tile_pool(name="sopool", bufs=8))
            qtzpool = ctx.enter_context(tc.tile_pool(name="qtzpool", bufs=2))

            # ---- static tiles
            mask_sb = consts.tile([128, 128], BF16)
            xT_sb = xpool.tile([128, NQC, NE, QC], BF16)
            v_sb = vpool.tile([128, NKB, 2 * GF], BF16)   # [tok, kb, h*(64V|64ones)]
            # q/k staged in bf16: the f32r score pair streams at 2 cyc/col
            # (fp32 moving-operand bandwidth); bf16 streams 1 cyc/col, so the
            # row-tiled pair really does cost ~width/2.4 ns.
            qts = [
                qkpool.tile([128, C], BF16, tag=f"qt{h}", name=f"qt{h}")
                for h in range(HP)
            ]
            kts = [
                qkpool.tile([128, C], BF16, tag=f"kt{h}", name=f"kt{h}")
                for h in range(HP)
            ]
            wqs = [
                wpool.tile([128, NE, 128], BF16, tag=f"wq{h}", name=f"wq{h}")
                for h in range(HP)
            ]
            wks = [
                wpool.tile([128, NE, 128], BF16, tag=f"wk{h}", name=f"wk{h}")
                for h in range(HP)
            ]
            wv_sb = wpool.tile([128, NE, GF], BF16, tag="wv")
            wo_sb = wpool.tile([128, HP, E], BF16, tag="wo")
            hf = hfpool.tile([128, HP, C], BF16)

            # ---- input DMAs.  DGE descriptor generation is ~0.45-0.9us of
            # sequencer time per dma_start, serial per sequencer.  x chunk 0
            # and wq0/wk0 are the critical path: chunk 0 is split in halves
            # across the sync and scalar sequencers so descriptor gen and the
            # two queue transfers overlap; everything else follows in
            # needed-by order.
            # the sync sequencer's DMA queue sustains ~356GB/s while the
            # scalar one ramps late and runs at ~100GB/s, so the whole
            # needed-early set goes through sync in consumption order; the
            # late x chunks ride the scalar queue to keep sync free for the
            # output DMAs
            # everything through the sync sequencer's queue (the scalar-
            # issued queue measures 4-6x slower), in strict consumption
            # order; ~8MB at ~356GB/s lands by ~31us, ahead of every
            # consumer, and input transfers finish before the first output
            # DMAs are issued
            nc.sync.dma_start(wqs[0][:], Wq.ap()[0])
            nc.sync.dma_start(mask_sb[:], msk.ap())
            nc.sync.dma_start(wks[0][:], Wk.ap()[0])
            nc.sync.dma_start(xT_sb[:, 0], xT_t[0])
            nc.sync.dma_start(wv_sb[:], Wv.ap())
            nc.sync.dma_start(wqs[1][:], Wq.ap()[1])
            nc.sync.dma_start(wks[1][:], Wk.ap()[1])
            nc.sync.dma_start(xT_sb[:, 1], xT_t[1])
            nc.sync.dma_start(wqs[2][:], Wq.ap()[2])
            nc.sync.dma_start(wks[2][:], Wk.ap()[2])
            nc.sync.dma_start(wqs[3][:], Wq.ap()[3])
            nc.sync.dma_start(wks[3][:], Wk.ap()[3])
            nc.sync.dma_start(wo_sb[:, 0:2, :], Wo.ap()[:, 0 : 2 * E])
            nc.sync.dma_start(wo_sb[:, 2:4, :], Wo.ap()[:, 2 * E : 4 * E])
            nc.sync.dma_start(xT_sb[:, 2], xT_t[2])
            nc.sync.dma_start(xT_sb[:, 3], xT_t[3])

            # ones columns for the rowsum trick.  Split: the first chunk's
            # k-blocks are needed ~10us in, the rest not before qc1 -- and a
            # single strided memset is ~7us of in-order DVE time that would
            # delay the early V casts.  The remainder is emitted a group
            # later (see the block loop).
            warm = consts.tile([128, 128], BF16, name="warm")
            nc.vector.memset(warm[:], 0.125)
            nc.gpsimd.memset(
                v_sb[:].rearrange("p kb (h u) -> p kb h u", u=128)[:, 0:4, :, 64:128],
                1.0,
            )
            # zero-padded q strips for the narrow boundary score blocks: a
            # full-height matmul against [q_h | zeros] avoids the row-grouped
            # pair's ~200ns weight-load transition stalls, which dominate at
            # width <= 384.  Strips are (re)written per (hp, qc) by the q
            # projection units; the zero halves are set once per buffer here.
            qtz_by = {}

            def qtz_tile(hp, n):
                if (hp, n) not in qtz_by:
                    qtz_by[(hp, n)] = qtzpool.tile(
                        [128, 2, 384], BF16, tag=f"qtz{hp}", name=f"qtz{hp}_{n}"
                    )
                return qtz_by[(hp, n)]

            for h in range(HP):
                for b in range(2):
                    t = qtz_tile(h, b - 2)      # pre-touch both ring buffers
                    nc.gpsimd.memset(t[64:128, 0, :], 0.0)
                    nc.gpsimd.memset(t[0:64, 1, :], 0.0)

            # warm the PE HAM clock gate while the engine preambles and the
            # input DMAs execute: sustained matmul activity flips the PE from
            # 1.2 to 2.4 GHz.  Warming on a locally-memset tile starts ~6us
            # earlier than anything DMA-fed.
            pp = pppool.tile([128, 1024], F32)        # shared proj/p3 accum
            for _ in range(64):
                nc.tensor.matmul(
                    pp[:, 0:128], lhsT=warm[:], rhs=warm[:],
                    start=True, stop=True,
                )

            # ---- filler machinery -------------------------------------
            # Each filler unit is a list of (pe_cost_ns, emit_fn) steps.
            # Units write alternating halves of the shared pp psum tile.
            pp_half = [0]

            def next_half():
                h = pp_half[0]
                pp_half[0] ^= 1
                return h

            def unit_v(kb):
                steps = []
                half = next_half()
                pv = pp[:, half * QC : (half + 1) * QC]
                for e in range(NE):
                    def mm(e=e, pv=pv, kb=kb):
                        nc.tensor.matmul(
                            pv,
                            lhsT=xT_sb[:, kb // 4, e, (kb % 4) * 128 : (kb % 4 + 1) * 128],
                            rhs=wv_sb[:, e, :],
                            start=(e == 0),
                            stop=(e == NE - 1),
                        )
                    steps.append((MM_NS, mm))
                def cp(pv=pv, kb=kb):
                    dst = v_sb[:, kb, :].rearrange("p (h u) -> p h u", u=128)[:, :, 0:64]
                    nc.vector.tensor_copy(dst, pv.rearrange("p (h u) -> p h u", u=64))
                steps.append((0.0, cp))
                return steps

            def unit_qk(which, hp, n):
                wt_, dst = (wqs[hp], qts[hp]) if which == "q" else (wks[hp], kts[hp])
                steps = []
                half = next_half()
                pq = pp[:, half * QC : (half + 1) * QC]
                for e in range(NE):
                    def mm(e=e, pq=pq, wt_=wt_, n=n):
                        nc.tensor.matmul(
                            pq,
                            lhsT=wt_[:, e, :],
                            rhs=xT_sb[:, n, e, :],
                            start=(e == 0),
                            stop=(e == NE - 1),
                        )
                    steps.append((MM_NS, mm))
                def cp(pq=pq, dst=dst, n=n, hp=hp):
                    # the first head-pair's q/k casts land before the first
                    # exp: run them on the idle ACT so the v0-3 casts lead
                    # the DVE queue (the first hidden matmuls wait on them)
                    if n == 0 and hp == 0:
                        nc.scalar.copy(dst[:, n * QC : (n + 1) * QC], pq)
                    else:
                        nc.vector.tensor_copy(dst[:, n * QC : (n + 1) * QC], pq)
                steps.append((0.0, cp))
                if which == "q":
                    def cpz(pq=pq, hp=hp, n=n):
                        t = qtz_tile(hp, n)
                        if n == 0 and hp == 0:
                            nc.scalar.copy(t[0:64, 0, :], pq[0:64, 128:QC])
                            nc.scalar.copy(t[64:128, 1, :], pq[64:128, 128:QC])
                        else:
                            nc.vector.tensor_copy(t[0:64, 0, :], pq[0:64, 128:QC])
                            nc.vector.tensor_copy(
                                t[64:128, 1, :], pq[64:128, 128:QC]
                            )
                    steps.append((0.0, cpz))
                return steps

            def unit_p3(qc, qb, ec):
                steps = []
                slot = next_half()
                def get_po(slot=slot):
                    return pp[:, slot * QC : (slot + 1) * QC]

                tok0 = qc * QC + qb * 128
                for f in range(HP):
                    def mm(f=f, tok0=tok0, ec=ec):
                        nc.tensor.matmul(
                            get_po(),
                            lhsT=hf[:, f, tok0 : tok0 + 128],
                            rhs=wo_sb[:, f, ec * QC : (ec + 1) * QC],
                            start=(f == 0),
                            stop=(f == HP - 1),
                        )
                    steps.append((MM_NS, mm))
                def cp(tok0=tok0, ec=ec):
                    so = sopool.tile([128, QC], BF16, tag="so")
                    nc.vector.tensor_copy(so[:], get_po())
                    nc.sync.dma_start(
                        out.ap()[tok0 : tok0 + 128, ec * QC : (ec + 1) * QC], so[:]
                    )
                steps.append((0.0, cp))
                return steps

            # ---- the final q-chunk's output projection is split: f0-f2
            # partial chains only need the first three head-pairs' hf (ready
            # mid-way through the last group), so they fill the ACT-bound
            # last-group stretch and the normalize latency in the drain; the
            # f3+store finals wait only on the very last normalize.  The 8
            # units hold their accumulators across the split in pp (2), two
            # st tiles (4) and the hid tile (2) -- all free by then.
            lastq_slots = {}

            def lastq_slot(u):
                if u not in lastq_slots:
                    if u < 2:
                        lastq_slots[0] = pp[:, 0:QC]
                        lastq_slots[1] = pp[:, QC : 2 * QC]
                    elif u < 4:
                        t = stpool.tile([128, 1024], F32, tag="st", name="p3st0")
                        lastq_slots[2] = t[:, 0:QC]
                        lastq_slots[3] = t[:, QC : 2 * QC]
                    elif u < 6:
                        t = stpool.tile([128, 1024], F32, tag="st", name="p3st1")
                        lastq_slots[4] = t[:, 0:QC]
                        lastq_slots[5] = t[:, QC : 2 * QC]
                    else:
                        t = hidpool.tile([128, 1024], F32, tag="hid", name="p3hid")
                        lastq_slots[6] = t[:, 0:QC]
                        lastq_slots[7] = t[:, QC : 2 * QC]
                return lastq_slots[u]

            def unit_p3_last_partial(u):
                steps = []
                qb, ec = u // 2, u % 2
                tok0 = (NQC - 1) * QC + qb * 128
                for f in range(3):
                    def mm(f=f, u=u, tok0=tok0, ec=ec):
                        nc.tensor.matmul(
                            lastq_slot(u),
                            lhsT=hf[:, f, tok0 : tok0 + 128],
                            rhs=wo_sb[:, f, ec * QC : (ec + 1) * QC],
                            start=(f == 0),
                            stop=False,
                            skip_group_check=True,
                        )
                    steps.append((MM_NS, mm))
                return steps

            def p3_last_final(u):
                qb, ec = u // 2, u % 2
                tok0 = (NQC - 1) * QC + qb * 128
                po = lastq_slot(u)
                nc.tensor.matmul(
                    po,
                    lhsT=hf[:, 3, tok0 : tok0 + 128],
                    rhs=wo_sb[:, 3, ec * QC : (ec + 1) * QC],
                    start=False,
                    stop=True,
                    skip_group_check=True,
                )
                so = sopool.tile([128, QC], BF16, tag="so")
                # split the drain copies across ACT (busy ~2.3us with the last
                # ln/exp) and DVE (busy ~2.5us with hraw+hf) so neither
                # serializes the final stores
                if u % 2 == 0:
                    nc.scalar.copy(so[:], po)
                else:
                    nc.vector.tensor_copy(so[:], po)
                nc.sync.dma_start(
                    out.ap()[tok0 : tok0 + 128, ec * QC : (ec + 1) * QC], so[:]
                )

            # ordered filler units with labels for prerequisite forcing
            fillers = []           # list of (label, steps)
            emitted = set()        # labels fully emitted
            cursor = [0, 0]        # (unit idx, step idx)
            debt = [0.0]

            def emit_steps_until(pred):
                ui, si = cursor
                while ui < len(fillers):
                    label, steps = fillers[ui]
                    while si < len(steps):
                        if pred():
                            cursor[0], cursor[1] = ui, si
                            return
                        cost, fn = steps[si]
                        fn()
                        debt[0] -= cost
                        si += 1
                    emitted.add(label)
                    ui += 1
                    si = 0
                cursor[0], cursor[1] = ui, si

            def pull_fillers():
                emit_steps_until(lambda: debt[0] <= 0.0)

            def ensure(labels):
                want = set(labels) - emitted
                if not want:
                    return
                emit_steps_until(lambda: not (set(labels) - emitted))
                missing = set(labels) - emitted
                assert not missing, f"filler order bug: {missing}"

            # filler order = consumption order of the attention groups, so
            # ensure() never force-drains far ahead of where it is needed
            for n in range(NQC):
                for hp in range(HP):
                    fillers.append((f"qk_q{hp}{n}", unit_qk("q", hp, n)))
                    fillers.append((f"qk_k{hp}{n}", unit_qk("k", hp, n)))
                    if hp == 0:
                        for kb in range(4 * n, 4 * n + 4):
                            fillers.append((f"v{kb}", unit_v(kb)))

            # ---- attention: one flat software-pipelined block stream ----
            # blocks from all (qc, hp) groups run as one stream; scores are
            # emitted one block ahead (across group boundaries too) so the
            # ACT engine streams exps back-to-back with no group bubbles.
            blocks = [
                (qc, hp, kb)
                for qc in range(NQC)
                for hp in range(HP)
                for kb in range(4 * qc + 4)
            ]

            def geom(qc, kb):
                dr = kb - 4 * qc
                c0 = 128 * dr if dr >= 0 else 0
                return dr, c0, QC - c0

            def emit_sc(qc, hp, kb):
                if kb == 0:
                    ensure([f"qk_q{hp}{n}" for n in range(qc + 1)]
                           + [f"qk_k{hp}{n}" for n in range(qc + 1)])
                qt, kt = qts[hp], kts[hp]
                dr, c0, width = geom(qc, kb)
                q0 = qc * QC + c0
                st = stpool.tile([128, 1024], F32, tag="st", name="st")
                if dr >= 1:
                    # narrow diagonal blocks: two full-height matmuls against
                    # the zero-padded q strip -- the extra streamed columns
                    # cost less than the row-grouped pair's LDWEIGHTS
                    # transition stalls at width <= 384
                    tz = qtz_tile(hp, qc)
                    off = (dr - 1) * 128
                    ktf = kt[:, kb * KB : (kb + 1) * KB]
                    nc.tensor.matmul(
                        st[:, 0:width], lhsT=ktf, rhs=tz[:, 0, off:384],
                        start=True, stop=True,
                    )
                    nc.tensor.matmul(
                        st[:, QC : QC + width], lhsT=ktf, rhs=tz[:, 1, off:384],
                        start=True, stop=True,
                    )
                    debt[0] -= 2 * width * 0.417 + 30.0
                else:
                    nc.tensor.matmul(
                        st[:, 0:width],
                        lhsT=kt[0:64, kb * KB : (kb + 1) * KB],
                        rhs=qt[0:64, q0 : (qc + 1) * QC],
                        start=True,
                        stop=True,
                    )
                    nc.tensor.matmul(
                        st[:, QC : QC + width],
                        lhsT=kt[64:128, kb * KB : (kb + 1) * KB],
                        rhs=qt[64:128, q0 : (qc + 1) * QC],
                        start=True,
                        stop=True,
                    )
                    debt[0] -= max(width * 0.417 + 15.0, 100.0)
                return st

            deferred = []   # pending normalize closure of the previous group
            p3_pending = []  # output-projection units held back for qc3
            hid = None
            st_next = emit_sc(0, 0, 0)
            debt[0] = 0.0   # prologue projections are PE head-start
            for i, (qc, hp, kb) in enumerate(blocks):
                dr, c0, width = geom(qc, kb)
                nkb = 4 * qc + 4
                st = st_next
                wt = wtpool.tile([128, 2, QC], BF16, tag="wt")
                nc.scalar.activation(
                    wt[:, :, 0:width],
                    st[:].rearrange("p (a b) -> p a b", a=2)[:, :, 0:width],
                    mybir.ActivationFunctionType.Exp,
                    scale=0.125,
                )
                debt[0] += 2 * width * 0.833 + 275
                if dr >= 0:
                    nc.vector.tensor_tensor(
                        wt[:, :, 0:128],
                        wt[:, :, 0:128],
                        mask_sb[:, None, :].to_broadcast((128, 2, 128)),
                        mybir.AluOpType.mult,
                    )
                if i + 1 < len(blocks):
                    st_next = emit_sc(*blocks[i + 1])
                if kb == (3 if qc == 0 else 5) and deferred:
                    deferred.pop()()
                if kb == (1 if qc == 0 else 2) and (qc, hp) != (NQC - 1, HP - 1):
                    # prefetch next group's q/k projections mid-group so
                    # their chains and copies finish before the boundary
                    # scores need them (qc0 groups are only 4 blocks, so
                    # prefetch a block earlier there)
                    nhp2 = (hp + 1) % HP
                    nqc2 = qc + 1 if nhp2 == 0 else qc
                    ensure([f"qk_q{nhp2}{n}" for n in range(nqc2 + 1)]
                           + [f"qk_k{nhp2}{n}" for n in range(nqc2 + 1)])
                if i == 4:
                    # ones for the remaining k-blocks (needed from qc1 on)
                    nc.vector.memset(
                        v_sb[:].rearrange("p kb (h u) -> p kb h u", u=128)[
                            :, 4:NKB, :, 64:128
                        ],
                        1.0,
                    )
                if (qc, hp) == (NQC - 1, HP - 1) and kb == 8:
                    # hf for head-pairs 0-2 of this chunk is complete (the
                    # (3,2) normalize popped at kb==5): the last chunk's
                    # first two p3 partial chains can fill this ACT-bound
                    # stretch
                    for u in range(2):
                        fillers.append((f"p3e{u}", unit_p3_last_partial(u)))
                ensure([f"v{kb}"])
                if kb + 1 < nkb:
                    # prefetch the next k-block's V unit so its psum->SBUF
                    # cast is done before the next block's hidden matmuls
                    ensure([f"v{kb + 1}"])
                pull_fillers()
                if kb == 0:
                    hid = hidpool.tile([128, 1024], F32, tag="hid", name="hid")
                for head in range(2):
                    nc.tensor.matmul(
                        hid[:, head * QC + c0 : (head + 1) * QC],
                        lhsT=v_sb[:, kb, (2 * hp + head) * 128 : (2 * hp + head + 1) * 128],
                        rhs=wt[:, head, 0:width],
                        start=(kb == 0),
                        stop=(kb == nkb - 1),
                        skip_group_check=True,
                    )
                    debt[0] -= MM_NS * width / QC
                if kb == nkb - 1:
                    # group done: free hid fast via raw bf16 copy to SBUF and
                    # defer the normalize (ln/exp/stage) into the next group
                    hraw = hrawpool.tile([128, 1024], BF16, tag="hraw", name="hraw")
                    nc.vector.tensor_copy(hraw[:], hid[:])

                    # All groups stage hf NEGATED (W_o is negated on the
                    # host to compensate): the DVE Newton chain below
                    # naturally converges to -1/rowsum, and per-group sign
                    # mixing would corrupt the shared output projection.
                    def normalize_act(qc=qc, hp=hp, hraw=hraw):
                        lnt = napool.tile([64, 1024], F32, tag="ln", name="lnt")
                        nc.scalar.activation(
                            lnt[:], hraw[64:128, :], mybir.ActivationFunctionType.Ln
                        )
                        rec = napool.tile([64, 1024], BF16, tag="rec", name="rec")
                        nc.scalar.activation(
                            rec[:], lnt[:],
                            mybir.ActivationFunctionType.Exp, scale=-1.0,
                        )
                        recn = napool.tile([64, 1024], BF16, tag="recn", name="recn")
                        nc.vector.tensor_scalar(
                            recn[:], rec[:], -1.0, None, mybir.AluOpType.mult
                        )
                        debt[0] += 2 * (1024 * 0.833 + 370)
                        qs = slice(qc * QC, (qc + 1) * QC)
                        nc.vector.tensor_tensor(
                            hf[0:64, hp, qs], hraw[0:64, 0:QC], recn[:, 0:QC],
                            mybir.AluOpType.mult,
                        )
                        nc.vector.tensor_tensor(
                            hf[64:128, hp, qs],
                            hraw[0:64, QC : 2 * QC], recn[:, QC : 2 * QC],
                            mybir.AluOpType.mult,
                        )

                    # qc3's exp stream is the local bottleneck (16 wide
                    # blocks, ~18us of ACT per group): the normalizes that
                    # would run inside it move to the DVE instead --
                    # Schraudolph-style seed (bitwise NOT of the bf16 bit
                    # pattern) plus two tuned Newton steps, ~0.4% max err,
                    # converging to -1/rowsum.
                    def normalize_dve(qc=qc, hp=hp, hraw=hraw):
                        rs0 = napool.tile([64, 1024], BF16, tag="rs0", name="rs0")
                        nc.vector.tensor_copy(rs0[:], hraw[64:128, :])
                        y0 = napool.tile([64, 1024], BF16, tag="y0", name="y0")
                        nc.vector.tensor_scalar(
                            y0[:].bitcast(mybir.dt.int16),
                            rs0[:].bitcast(mybir.dt.int16),
                            0, None, mybir.AluOpType.bitwise_not,
                        )
                        y0f = napool.tile([64, 1024], F32, tag="y0f", name="y0f")
                        nc.vector.tensor_scalar(
                            y0f[:], y0[:], -0.23549792, None, mybir.AluOpType.mult
                        )
                        a1 = napool.tile([64, 1024], F32, tag="nra", name="nra1")
                        nc.vector.tensor_tensor(
                            a1[:], rs0[:], y0f[:], mybir.AluOpType.mult
                        )
                        z1 = napool.tile([64, 1024], F32, tag="nz1", name="nz1")
                        nc.vector.scalar_tensor_tensor(
                            z1[:], a1[:], 2.0017324, y0f[:],
                            mybir.AluOpType.subtract, mybir.AluOpType.mult,
                        )
                        a2 = napool.tile([64, 1024], F32, tag="nra", name="nra2")
                        nc.vector.tensor_tensor(
                            a2[:], rs0[:], z1[:], mybir.AluOpType.mult
                        )
                        z2 = napool.tile([64, 1024], F32, tag="nz2", name="nz2")
                        nc.vector.scalar_tensor_tensor(
                            z2[:], a2[:], 2.0, z1[:],
                            mybir.AluOpType.add, mybir.AluOpType.mult,
                        )
                        qs = slice(qc * QC, (qc + 1) * QC)
                        nc.vector.tensor_tensor(
                            hf[0:64, hp, qs], hraw[0:64, 0:QC], z2[:, 0:QC],
                            mybir.AluOpType.mult,
                        )
                        nc.vector.tensor_tensor(
                            hf[64:128, hp, qs],
                            hraw[0:64, QC : 2 * QC], z2[:, QC : 2 * QC],
                            mybir.AluOpType.mult,
                        )

                    # (3,2)'s normalize gates the drain's p3 partials: keep
                    # it on ACT (low latency); the DVE chain's ~7us latency
                    # only suits groups deep inside the qc3 exp stream
                    use_dve = (qc, hp) in ((2, 3), (3, 0), (3, 1))
                    deferred.append(normalize_dve if use_dve else normalize_act)
                    if hp == HP - 1 and qc < NQC - 1:
                        # this q-chunk's output projection becomes filler --
                        # but hold it back until qc2 is done, so the final
                        # (filler-starved) q-chunk has PE work to hide its
                        # exp latencies behind
                        for qb in range(4):
                            for ec in range(E // QC):
                                fillers.append(
                                    (f"p3_{qc}_{qb}_{ec}", unit_p3(qc, qb, ec))
                                )

            # drain: flush the last normalize (ACT+DVE), overlap it with the
            # remaining p3 partial chains on the PE, then the f3+store finals
            while deferred:
                deferred.pop()()
            for u in range(2, 8):
                for _cost, fn in unit_p3_last_partial(u):
                    fn()
            emit_steps_until(lambda: False)
            for u in range(8):
                p3_last_final(u)
    return nc


def _make_mask():
    import ml_dtypes

    kk = np.arange(128)[:, None]
    cc = np.arange(128)[None, :]
    return (kk <= cc).astype(np.float32).astype(ml_dtypes.bfloat16)


def make_in_maps(x, W_q, W_k, W_v, W_o):
    import ml_dtypes

    bf16 = ml_dtypes.bfloat16
    mask = _make_mask()
    in_maps = []
    def repack_qk(w):
        # [E, GF] -> [hp, pi, po*128]: contiguous per partition line
        return np.ascontiguousarray(
            w.reshape(NE, 128, HP, 128).transpose(2, 1, 0, 3).reshape(HP, 128, -1)
        )

    def repack_o(w):
        # [GF, E] -> [pi, po*E]: contiguous per partition line.  Negated:
        # the kernel stages hf as -hidden/rowsum (the DVE reciprocal chain
        # converges to -1/rowsum), so (-hf) @ (-W_o) restores the sign.
        return np.ascontiguousarray(
            -w.reshape(HP, 128, E).transpose(1, 0, 2).reshape(128, -1)
        )

    def repack_x(xb):
        # [C, E] -> [NQC, 128, NE, QC]: per q-chunk, contiguous per
        # partition line (one descriptor-light dma_start per chunk)
        return np.ascontiguousarray(
            xb.reshape(NQC, QC, NE, 128).transpose(0, 3, 2, 1)
        )

    def repack_v(w):
        # [E, GF] -> [128, NE, GF]: contiguous per partition line
        return np.ascontiguousarray(w.reshape(NE, 128, GF).transpose(1, 0, 2))

    for i in range(N_CORES):
        b, g = i // 2, i % 2
        in_maps.append(
            {
                "xT": repack_x(np.asarray(x)[b]).astype(bf16),
                "Wq": repack_qk(np.asarray(W_q)[:, g * GF : (g + 1) * GF]).astype(bf16),
                "Wk": repack_qk(np.asarray(W_k)[:, g * GF : (g + 1) * GF]).astype(bf16),
                "Wv": repack_v(np.asarray(W_v)[:, g * GF : (g + 1) * GF]).astype(bf16),
                "Wo": repack_o(np.asarray(W_o)[g * GF : (g + 1) * GF, :]).astype(bf16),
                "mask": mask,
            }
        )
    return in_maps


def kernel(x, W_q, W_k, W_v, W_o):
    global _CACHED_NC
    from concourse.bass_utils import run_bass_kernel_spmd

    if _CACHED_NC is None:
        _CACHED_NC = build_nc()
    nc = _CACHED_NC

    in_maps = make_in_maps(x, W_q, W_k, W_v, W_o)
    res = run_bass_kernel_spmd(nc, in_maps, core_ids=list(range(N_CORES)))
    out = np.empty((B, C, E), dtype=np.float32)
    for b in range(B):
        out[b] = np.asarray(res.results[2 * b]["out"], dtype=np.float32) + np.asarray(
            res.results[2 * b + 1]["out"], dtype=np.float32
        )
    return out



# revision 35
# speedup vs baseline: 1.0684x; 1.0684x over previous
"""Multi-head causal attention (B=4, C=2048, E=1024, H=16, D=64) on 8 trn2 cores.

Sharding: core i = (batch b=i//2, head-group g=i%2).  Each core computes its
batch's attention for 8 heads (512 features) and a partial output projection;
the host sums the two partials per batch (W_o split row-wise).

Single fused pipeline per core:
  - qc-major attention (hp inner) with the output projection for q-chunk qc
    emitted as PE filler inside q-chunk qc+1 -- hidden never round-trips
    through DRAM.
  - V / Q / K projection chains are *fillers*: emitted between attention
    blocks under a credit scheduler so the in-order PE queue never idles
    behind the score->exp->hidden dependency chain.
  - q/k staged in BF16: fp32(r) moving operands stream at 2 cyc/col, so the
    row-tiled score pair really costs ~width/2.4 ns in bf16 (half of f32r).
  - diagonal blocks are trimmed: only q-columns >= k are computed (scores,
    exp, hidden all shrink); the causal mask is one 128x128 triangle applied
    to the boundary strip only.
  - PSUM: st[128,1024]x2 + hid[128,1024]x1 + pp[128,1024]x1 = 8 banks.
    hid is freed via one bf16 copy to SBUF.  1/rowsum: exp(-ln) on ACT for
    most groups; a Schraudolph+2-Newton DVE chain (yielding -1/rowsum; W_o
    is negated on the host) for the groups whose normalize would land in
    the ACT-bound qc3 exp stream.
  - all inputs ride the sync sequencer's DMA queue (~356GB/s; the scalar
    one is 4-6x slower) in consumption order, descriptor-minimal host
    repacks (one dma_start per q-chunk / weight block).
  - the PE warms its HAM clock gate on a locally-memset tile from ~6.5us
    (no DMA dependency); the final q-chunk's output projection is split
    f0-f2 / f3+store so only the last f3 matmuls wait on the last
    normalize.
"""

import numpy as np

import concourse.bass as bass
import concourse.mybir as mybir
import concourse.tile as tile
from concourse.vector_clock import ScopedClock

B, C, E = 4, 2048, 1024
H, D = 16, 64
N_CORES = 8
GF = 512          # features per head-group (8 heads x 64)
HP = 4            # head-pairs per group
QC = 512          # q-chunk width
KB = 128          # k-block width
NQC = C // QC     # 4
NKB = C // KB     # 16
NE = E // 128     # 8 contraction tiles over E
F32 = mybir.dt.float32
F32R = mybir.dt.float32r
BF16 = mybir.dt.bfloat16

_CACHED_NC = None


class PatchedTC(tile.TileContext):
    """This walrus build caps sync waits per instruction (1 for CTRL, ~2 for
    compute ISA structs).  Hoist excess waits onto same-engine NOPs emitted
    just before the instruction (engine streams execute in order, so the
    semantics are identical), and split the end-of-kernel drain's waits
    across single-wait drain instructions."""

    WAIT_CAP = 1

    def _commit_instruction(self, inst, lazy_reg_writes=True):
        si = getattr(inst, "sync_info", None)
        if (
            si is not None
            and len(si.on_wait) > self.WAIT_CAP
            and getattr(inst, "engine", mybir.EngineType.Unassigned)
            != mybir.EngineType.Unassigned
        ):
            waits = list(si.on_wait)
            keep = waits[: self.WAIT_CAP]
            extra = waits[self.WAIT_CAP :]
            si.on_wait[:] = keep
            for w in extra:
                nop = mybir.InstNoOp(
                    name=f"I-nw{self.nc.next_id()}",
                    engine=inst.engine,
                    bass_nofuse=True,
                    sync_info=mybir.SyncInfo(on_wait=[w], on_update=[]),
                )
                super()._commit_instruction(nop, lazy_reg_writes=False)
        return super()._commit_instruction(inst, lazy_reg_writes)

    def _drain_and_barrier(self, tick_clock, wait_clock):
        carrier = self.nc.sync.drain()
        wait_clock.add_sem_waits(
            carrier.ins, ScopedClock({None: tick_clock.global_clock})
        )
        si = carrier.ins.sync_info
        waits = list(si.on_wait) if si is not None else []
        if len(waits) > 1:
            si.on_wait[:] = waits[:1]
            for w in waits[1:]:
                extra = self.nc.sync.drain()
                extra.ins.sync_info = mybir.SyncInfo(on_wait=[w], on_update=[])
        self.nc.all_engine_barrier()
        assert self.sems is not None
        popped = self.nc._tile_sem_poison_stack.pop()
        assert popped is self._sem_poison
        self.nc.clear_and_free_semaphores(list(self.sems.allocated().values()))
        self.nc.all_engine_barrier()


def build_nc():
    nc = bass.Bass("TRN2", target_bir_lowering=False)
    # x is repacked chunk-major on the host ([NQC, 128, NE, QC], partition-
    # line contiguous) so one q-chunk loads with a single dma_start
    # (~128 descriptors) instead of 8 strided ones.
    xT = nc.declare_dram_parameter("xT", [NQC, 128, NE, QC], BF16, isOutput=False)
    # W_q/W_k/W_v/W_o are repacked on the host into descriptor-minimal
    # layouts (contiguous per partition line) -- DGE generation is
    # ~7ns/descriptor, so the default strided layouts cost 2.5-3.6us of
    # sequencer time each
    Wq = nc.declare_dram_parameter("Wq", [HP, 128, NE * 128], BF16, isOutput=False)
    Wk = nc.declare_dram_parameter("Wk", [HP, 128, NE * 128], BF16, isOutput=False)
    Wv = nc.declare_dram_parameter("Wv", [128, NE, GF], BF16, isOutput=False)
    Wo = nc.declare_dram_parameter("Wo", [128, HP * E], BF16, isOutput=False)
    msk = nc.declare_dram_parameter("mask", [128, 128], BF16, isOutput=False)
    out = nc.declare_dram_parameter("out", [C, E], BF16, isOutput=True)

    xT_t = xT.ap()                                  # [NQC, 128, NE, QC]

    MM_NS = 216.0        # back-to-back bf16 N=512 matmul
    PAIR_NS = 228.0      # concurrent bf16 row-tiled pair, N=512

    with PatchedTC(nc) as tc:
        import contextlib

        with contextlib.ExitStack() as ctx:
            consts = ctx.enter_context(tc.tile_pool(name="consts", bufs=1))
            xpool = ctx.enter_context(tc.tile_pool(name="xpool", bufs=1))
            vpool = ctx.enter_context(tc.tile_pool(name="vpool", bufs=1))
            qkpool = ctx.enter_context(tc.tile_pool(name="qkpool", bufs=1))
            wpool = ctx.enter_context(tc.tile_pool(name="wpool", bufs=1))
            hfpool = ctx.enter_context(tc.tile_pool(name="hfpool", bufs=1))
            stpool = ctx.enter_context(tc.tile_pool(name="stp", bufs=2, space="PSUM"))
            hidpool = ctx.enter_context(tc.tile_pool(name="hidp", bufs=1, space="PSUM"))
            pppool = ctx.enter_context(tc.tile_pool(name="ppp", bufs=1, space="PSUM"))
            wtpool = ctx.enter_context(tc.tile_pool(name="wtpool", bufs=3))
            hrawpool = ctx.enter_context(tc.tile_pool(name="hrawpool", bufs=2))
            napool = ctx.enter_context(tc.tile_pool(name="napool", bufs=1))
            sopool = ctx.enter_context(tc.tile_pool(name="sopool", bufs=8))

            # ---- static tiles
            mask_sb = consts.tile([128, 128], BF16)
            xT_sb = xpool.tile([128, NQC, NE, QC], BF16)
            v_sb = vpool.tile([128, NKB, 2 * GF], BF16)   # [tok, kb, h*(64V|64ones)]
            # q/k staged in bf16: the f32r score pair streams at 2 cyc/col
            # (fp32 moving-operand bandwidth); bf16 streams 1 cyc/col, so the
            # row-tiled pair really does cost ~width/2.4 ns.
            qts = [
                qkpool.tile([128, C], BF16, tag=f"qt{h}", name=f"qt{h}")
                for h in range(HP)
            ]
            kts = [
                qkpool.tile([128, C], BF16, tag=f"kt{h}", name=f"kt{h}")
                for h in range(HP)
            ]
            wqs = [
                wpool.tile([128, NE, 128], BF16, tag=f"wq{h}", name=f"wq{h}")
                for h in range(HP)
            ]
            wks = [
                wpool.tile([128, NE, 128], BF16, tag=f"wk{h}", name=f"wk{h}")
                for h in range(HP)
            ]
            wv_sb = wpool.tile([128, NE, GF], BF16, tag="wv")
            wo_sb = wpool.tile([128, HP, E], BF16, tag="wo")
            hf = hfpool.tile([128, HP, C], BF16)

            # ---- input DMAs.  DGE descriptor generation is ~0.45-0.9us of
            # sequencer time per dma_start, serial per sequencer.  x chunk 0
            # and wq0/wk0 are the critical path: chunk 0 is split in halves
            # across the sync and scalar sequencers so descriptor gen and the
            # two queue transfers overlap; everything else follows in
            # needed-by order.
            # the sync sequencer's DMA queue sustains ~356GB/s while the
            # scalar one ramps late and runs at ~100GB/s, so the whole
            # needed-early set goes through sync in consumption order; the
            # late x chunks ride the scalar queue to keep sync free for the
            # output DMAs
            # everything through the sync sequencer's queue (the scalar-
            # issued queue measures 4-6x slower), in strict consumption
            # order; ~8MB at ~356GB/s lands by ~31us, ahead of every
            # consumer, and input transfers finish before the first output
            # DMAs are issued
            nc.sync.dma_start(wqs[0][:], Wq.ap()[0])
            nc.sync.dma_start(mask_sb[:], msk.ap())
            nc.sync.dma_start(wks[0][:], Wk.ap()[0])
            nc.sync.dma_start(xT_sb[:, 0], xT_t[0])
            nc.sync.dma_start(wv_sb[:], Wv.ap())
            nc.sync.dma_start(wqs[1][:], Wq.ap()[1])
            nc.sync.dma_start(wks[1][:], Wk.ap()[1])
            nc.sync.dma_start(xT_sb[:, 1], xT_t[1])
            nc.sync.dma_start(wqs[2][:], Wq.ap()[2])
            nc.sync.dma_start(wks[2][:], Wk.ap()[2])
            nc.sync.dma_start(wqs[3][:], Wq.ap()[3])
            nc.sync.dma_start(wks[3][:], Wk.ap()[3])
            nc.sync.dma_start(wo_sb[:, 0:2, :], Wo.ap()[:, 0 : 2 * E])
            nc.sync.dma_start(wo_sb[:, 2:4, :], Wo.ap()[:, 2 * E : 4 * E])
            nc.sync.dma_start(xT_sb[:, 2], xT_t[2])
            nc.sync.dma_start(xT_sb[:, 3], xT_t[3])

            # ones columns for the rowsum trick.  Split: the first chunk's
            # k-blocks are needed ~10us in, the rest not before qc1 -- and a
            # single strided memset is ~7us of in-order DVE time that would
            # delay the early V casts.  The remainder is emitted a group
            # later (see the block loop).
            warm = consts.tile([128, 128], BF16, name="warm")
            nc.vector.memset(warm[:], 0.125)
            nc.gpsimd.memset(
                v_sb[:].rearrange("p kb (h u) -> p kb h u", u=128)[:, 0:4, :, 64:128],
                1.0,
            )

            # warm the PE HAM clock gate while the engine preambles and the
            # input DMAs execute: sustained matmul activity flips the PE from
            # 1.2 to 2.4 GHz.  Warming on a locally-memset tile starts ~6us
            # earlier than anything DMA-fed.
            pp = pppool.tile([128, 1024], F32)        # shared proj/p3 accum
            for _ in range(64):
                nc.tensor.matmul(
                    pp[:, 0:128], lhsT=warm[:], rhs=warm[:],
                    start=True, stop=True,
                )

            # ---- filler machinery -------------------------------------
            # Each filler unit is a list of (pe_cost_ns, emit_fn) steps.
            # Units write alternating halves of the shared pp psum tile.
            pp_half = [0]

            def next_half():
                h = pp_half[0]
                pp_half[0] ^= 1
                return h

            def unit_v(kb):
                steps = []
                half = next_half()
                pv = pp[:, half * QC : (half + 1) * QC]
                for e in range(NE):
                    def mm(e=e, pv=pv, kb=kb):
                        nc.tensor.matmul(
                            pv,
                            lhsT=xT_sb[:, kb // 4, e, (kb % 4) * 128 : (kb % 4 + 1) * 128],
                            rhs=wv_sb[:, e, :],
                            start=(e == 0),
                            stop=(e == NE - 1),
                        )
                    steps.append((MM_NS, mm))
                def cp(pv=pv, kb=kb):
                    dst = v_sb[:, kb, :].rearrange("p (h u) -> p h u", u=128)[:, :, 0:64]
                    nc.vector.tensor_copy(dst, pv.rearrange("p (h u) -> p h u", u=64))
                steps.append((0.0, cp))
                return steps

            def unit_qk(which, hp, n):
                wt_, dst = (wqs[hp], qts[hp]) if which == "q" else (wks[hp], kts[hp])
                steps = []
                half = next_half()
                pq = pp[:, half * QC : (half + 1) * QC]
                for e in range(NE):
                    def mm(e=e, pq=pq, wt_=wt_, n=n):
                        nc.tensor.matmul(
                            pq,
                            lhsT=wt_[:, e, :],
                            rhs=xT_sb[:, n, e, :],
                            start=(e == 0),
                            stop=(e == NE - 1),
                        )
                    steps.append((MM_NS, mm))
                def cp(pq=pq, dst=dst, n=n, hp=hp):
                    # the first head-pair's q/k casts land before the first
                    # exp: run them on the idle ACT so the v0-3 casts lead
                    # the DVE queue (the first hidden matmuls wait on them)
                    if n == 0 and hp == 0:
                        nc.scalar.copy(dst[:, n * QC : (n + 1) * QC], pq)
                    else:
                        nc.vector.tensor_copy(dst[:, n * QC : (n + 1) * QC], pq)
                steps.append((0.0, cp))
                return steps

            def unit_p3(qc, qb, ec):
                steps = []
                slot = next_half()
                def get_po(slot=slot):
                    return pp[:, slot * QC : (slot + 1) * QC]

                tok0 = qc * QC + qb * 128
                for f in range(HP):
                    def mm(f=f, tok0=tok0, ec=ec):
                        nc.tensor.matmul(
                            get_po(),
                            lhsT=hf[:, f, tok0 : tok0 + 128],
                            rhs=wo_sb[:, f, ec * QC : (ec + 1) * QC],
                            start=(f == 0),
                            stop=(f == HP - 1),
                        )
                    steps.append((MM_NS, mm))
                def cp(tok0=tok0, ec=ec):
                    so = sopool.tile([128, QC], BF16, tag="so")
                    nc.vector.tensor_copy(so[:], get_po())
                    nc.sync.dma_start(
                        out.ap()[tok0 : tok0 + 128, ec * QC : (ec + 1) * QC], so[:]
                    )
                steps.append((0.0, cp))
                return steps

            # ---- the final q-chunk's output projection is split: f0-f2
            # partial chains only need the first three head-pairs' hf (ready
            # mid-way through the last group), so they fill the ACT-bound
            # last-group stretch and the normalize latency in the drain; the
            # f3+store finals wait only on the very last normalize.  The 8
            # units hold their accumulators across the split in pp (2), two
            # st tiles (4) and the hid tile (2) -- all free by then.
            lastq_slots = {}

            def lastq_slot(u):
                if u not in lastq_slots:
                    if u < 2:
                        lastq_slots[0] = pp[:, 0:QC]
                        lastq_slots[1] = pp[:, QC : 2 * QC]
                    elif u < 4:
                        t = stpool.tile([128, 1024], F32, tag="st", name="p3st0")
                        lastq_slots[2] = t[:, 0:QC]
                        lastq_slots[3] = t[:, QC : 2 * QC]
                    elif u < 6:
                        t = stpool.tile([128, 1024], F32, tag="st", name="p3st1")
                        lastq_slots[4] = t[:, 0:QC]
                        lastq_slots[5] = t[:, QC : 2 * QC]
                    else:
                        t = hidpool.tile([128, 1024], F32, tag="hid", name="p3hid")
                        lastq_slots[6] = t[:, 0:QC]
                        lastq_slots[7] = t[:, QC : 2 * QC]
                return lastq_slots[u]

            def unit_p3_last_partial(u):
                steps = []
                qb, ec = u // 2, u % 2
                tok0 = (NQC - 1) * QC + qb * 128
                for f in range(3):
                    def mm(f=f, u=u, tok0=tok0, ec=ec):
                        nc.tensor.matmul(
                            lastq_slot(u),
                            lhsT=hf[:, f, tok0 : tok0 + 128],
                            rhs=wo_sb[:, f, ec * QC : (ec + 1) * QC],
                            start=(f == 0),
                            stop=False,
                            skip_group_check=True,
                        )
                    steps.append((MM_NS, mm))
                return steps

            def p3_last_final(u):
                qb, ec = u // 2, u % 2
                tok0 = (NQC - 1) * QC + qb * 128
                po = lastq_slot(u)
                nc.tensor.matmul(
                    po,
                    lhsT=hf[:, 3, tok0 : tok0 + 128],
                    rhs=wo_sb[:, 3, ec * QC : (ec + 1) * QC],
                    start=False,
                    stop=True,
                    skip_group_check=True,
                )
                so = sopool.tile([128, QC], BF16, tag="so")
                # split the drain copies across ACT (busy ~2.3us with the last
                # ln/exp) and DVE (busy ~2.5us with hraw+hf) so neither
                # serializes the final stores
                if u % 2 == 0:
                    nc.scalar.copy(so[:], po)
                else:
                    nc.vector.tensor_copy(so[:], po)
                nc.sync.dma_start(
                    out.ap()[tok0 : tok0 + 128, ec * QC : (ec + 1) * QC], so[:]
                )

            # ordered filler units with labels for prerequisite forcing
            fillers = []           # list of (label, steps)
            emitted = set()        # labels fully emitted
            cursor = [0, 0]        # (unit idx, step idx)
            debt = [0.0]

            def emit_steps_until(pred):
                ui, si = cursor
                while ui < len(fillers):
                    label, steps = fillers[ui]
                    while si < len(steps):
                        if pred():
                            cursor[0], cursor[1] = ui, si
                            return
                        cost, fn = steps[si]
                        fn()
                        debt[0] -= cost
                        si += 1
                    emitted.add(label)
                    ui += 1
                    si = 0
                cursor[0], cursor[1] = ui, si

            def pull_fillers():
                emit_steps_until(lambda: debt[0] <= 0.0)

            def ensure(labels):
                want = set(labels) - emitted
                if not want:
                    return
                emit_steps_until(lambda: not (set(labels) - emitted))
                missing = set(labels) - emitted
                assert not missing, f"filler order bug: {missing}"

            # filler order = consumption order of the attention groups, so
            # ensure() never force-drains far ahead of where it is needed
            for n in range(NQC):
                for hp in range(HP):
                    fillers.append((f"qk_q{hp}{n}", unit_qk("q", hp, n)))
                    fillers.append((f"qk_k{hp}{n}", unit_qk("k", hp, n)))
                    if hp == 0:
                        for kb in range(4 * n, 4 * n + 4):
                            fillers.append((f"v{kb}", unit_v(kb)))

            # ---- attention: one flat software-pipelined block stream ----
            # blocks from all (qc, hp) groups run as one stream; scores are
            # emitted one block ahead (across group boundaries too) so the
            # ACT engine streams exps back-to-back with no group bubbles.
            blocks = [
                (qc, hp, kb)
                for qc in range(NQC)
                for hp in range(HP)
                for kb in range(4 * qc + 4)
            ]

            def geom(qc, kb):
                dr = kb - 4 * qc
                c0 = 128 * dr if dr >= 0 else 0
                return dr, c0, QC - c0

            def emit_sc(qc, hp, kb):
                if kb == 0:
                    ensure([f"qk_q{hp}{n}" for n in range(qc + 1)]
                           + [f"qk_k{hp}{n}" for n in range(qc + 1)])
                qt, kt = qts[hp], kts[hp]
                dr, c0, width = geom(qc, kb)
                q0 = qc * QC + c0
                st = stpool.tile([128, 1024], F32, tag="st", name="st")
                nc.tensor.matmul(
                    st[:, 0:width],
                    lhsT=kt[0:64, kb * KB : (kb + 1) * KB],
                    rhs=qt[0:64, q0 : (qc + 1) * QC],
                    start=True,
                    stop=True,
                )
                nc.tensor.matmul(
                    st[:, QC : QC + width],
                    lhsT=kt[64:128, kb * KB : (kb + 1) * KB],
                    rhs=qt[64:128, q0 : (qc + 1) * QC],
                    start=True,
                    stop=True,
                )
                debt[0] -= max(width * 0.417 + 15.0, 100.0)
                return st

            deferred = []   # pending normalize closure of the previous group
            p3_pending = []  # output-projection units held back for qc3
            hid = None
            st_next = emit_sc(0, 0, 0)
            debt[0] = 0.0   # prologue projections are PE head-start
            for i, (qc, hp, kb) in enumerate(blocks):
                dr, c0, width = geom(qc, kb)
                nkb = 4 * qc + 4
                st = st_next
                wt = wtpool.tile([128, 2, QC], BF16, tag="wt")
                nc.scalar.activation(
                    wt[:, :, 0:width],
                    st[:].rearrange("p (a b) -> p a b", a=2)[:, :, 0:width],
                    mybir.ActivationFunctionType.Exp,
                    scale=0.125,
                )
                debt[0] += 2 * width * 0.833 + 275
                if dr >= 0:
                    nc.vector.tensor_tensor(
                        wt[:, :, 0:128],
                        wt[:, :, 0:128],
                        mask_sb[:, None, :].to_broadcast((128, 2, 128)),
                        mybir.AluOpType.mult,
                    )
                if i + 1 < len(blocks):
                    st_next = emit_sc(*blocks[i + 1])
                if kb == (3 if qc == 0 else 5) and deferred:
                    deferred.pop()()
                if kb == (1 if qc == 0 else 2) and (qc, hp) != (NQC - 1, HP - 1):
                    # prefetch next group's q/k projections mid-group so
                    # their chains and copies finish before the boundary
                    # scores need them (qc0 groups are only 4 blocks, so
                    # prefetch a block earlier there)
                    nhp2 = (hp + 1) % HP
                    nqc2 = qc + 1 if nhp2 == 0 else qc
                    ensure([f"qk_q{nhp2}{n}" for n in range(nqc2 + 1)]
                           + [f"qk_k{nhp2}{n}" for n in range(nqc2 + 1)])
                if i == 4:
                    # ones for the remaining k-blocks (needed from qc1 on)
                    nc.vector.memset(
                        v_sb[:].rearrange("p kb (h u) -> p kb h u", u=128)[
                            :, 4:NKB, :, 64:128
                        ],
                        1.0,
                    )
                if (qc, hp) == (NQC - 1, HP - 1) and kb == 8:
                    # hf for head-pairs 0-2 of this chunk is complete (the
                    # (3,2) normalize popped at kb==5): the last chunk's
                    # first two p3 partial chains can fill this ACT-bound
                    # stretch
                    for u in range(2):
                        fillers.append((f"p3e{u}", unit_p3_last_partial(u)))
                ensure([f"v{kb}"])
                if kb + 1 < nkb:
                    # prefetch the next k-block's V unit so its psum->SBUF
                    # cast is done before the next block's hidden matmuls
                    ensure([f"v{kb + 1}"])
                pull_fillers()
                if kb == 0:
                    hid = hidpool.tile([128, 1024], F32, tag="hid", name="hid")
                for head in range(2):
                    nc.tensor.matmul(
                        hid[:, head * QC + c0 : (head + 1) * QC],
                        lhsT=v_sb[:, kb, (2 * hp + head) * 128 : (2 * hp + head + 1) * 128],
                        rhs=wt[:, head, 0:width],
                        start=(kb == 0),
                        stop=(kb == nkb - 1),
                        skip_group_check=True,
                    )
                    debt[0] -= MM_NS * width / QC
                if kb == nkb - 1:
                    # group done: free hid fast via raw bf16 copy to SBUF and
                    # defer the normalize (ln/exp/stage) into the next group
                    hraw = hrawpool.tile([128, 1024], BF16, tag="hraw", name="hraw")
                    nc.vector.tensor_copy(hraw[:], hid[:])

                    # All groups stage hf NEGATED (W_o is negated on the
                    # host to compensate): the DVE Newton chain below
                    # naturally converges to -1/rowsum, and per-group sign
                    # mixing would corrupt the shared output projection.
                    def normalize_act(qc=qc, hp=hp, hraw=hraw):
                        lnt = napool.tile([64, 1024], F32, tag="ln", name="lnt")
                        nc.scalar.activation(
                            lnt[:], hraw[64:128, :], mybir.ActivationFunctionType.Ln
                        )
                        rec = napool.tile([64, 1024], BF16, tag="rec", name="rec")
                        nc.scalar.activation(
                            rec[:], lnt[:],
                            mybir.ActivationFunctionType.Exp, scale=-1.0,
                        )
                        recn = napool.tile([64, 1024], BF16, tag="recn", name="recn")
                        nc.vector.tensor_scalar(
                            recn[:], rec[:], -1.0, None, mybir.AluOpType.mult
                        )
                        debt[0] += 2 * (1024 * 0.833 + 370)
                        qs = slice(qc * QC, (qc + 1) * QC)
                        nc.vector.tensor_tensor(
                            hf[0:64, hp, qs], hraw[0:64, 0:QC], recn[:, 0:QC],
                            mybir.AluOpType.mult,
                        )
                        nc.vector.tensor_tensor(
                            hf[64:128, hp, qs],
                            hraw[0:64, QC : 2 * QC], recn[:, QC : 2 * QC],
                            mybir.AluOpType.mult,
                        )

                    # qc3's exp stream is the local bottleneck (16 wide
                    # blocks, ~18us of ACT per group): the normalizes that
                    # would run inside it move to the DVE instead --
                    # Schraudolph-style seed (bitwise NOT of the bf16 bit
                    # pattern) plus two tuned Newton steps, ~0.4% max err,
                    # converging to -1/rowsum.
                    def normalize_dve(qc=qc, hp=hp, hraw=hraw):
                        rs0 = napool.tile([64, 1024], BF16, tag="rs0", name="rs0")
                        nc.vector.tensor_copy(rs0[:], hraw[64:128, :])
                        y0 = napool.tile([64, 1024], BF16, tag="y0", name="y0")
                        nc.vector.tensor_scalar(
                            y0[:].bitcast(mybir.dt.int16),
                            rs0[:].bitcast(mybir.dt.int16),
                            0, None, mybir.AluOpType.bitwise_not,
                        )
                        y0f = napool.tile([64, 1024], F32, tag="y0f", name="y0f")
                        nc.vector.tensor_scalar(
                            y0f[:], y0[:], -0.23549792, None, mybir.AluOpType.mult
                        )
                        a1 = napool.tile([64, 1024], F32, tag="nra", name="nra1")
                        nc.vector.tensor_tensor(
                            a1[:], rs0[:], y0f[:], mybir.AluOpType.mult
                        )
                        z1 = napool.tile([64, 1024], F32, tag="nz1", name="nz1")
                        nc.vector.scalar_tensor_tensor(
                            z1[:], a1[:], 2.0017324, y0f[:],
                            mybir.AluOpType.subtract, mybir.AluOpType.mult,
                        )
                        a2 = napool.tile([64, 1024], F32, tag="nra", name="nra2")
                        nc.vector.tensor_tensor(
                            a2[:], rs0[:], z1[:], mybir.AluOpType.mult
                        )
                        z2 = napool.tile([64, 1024], F32, tag="nz2", name="nz2")
                        nc.vector.scalar_tensor_tensor(
                            z2[:], a2[:], 2.0, z1[:],
                            mybir.AluOpType.add, mybir.AluOpType.mult,
                        )
                        qs = slice(qc * QC, (qc + 1) * QC)
                        nc.vector.tensor_tensor(
                            hf[0:64, hp, qs], hraw[0:64, 0:QC], z2[:, 0:QC],
                            mybir.AluOpType.mult,
                        )
                        nc.vector.tensor_tensor(
                            hf[64:128, hp, qs],
                            hraw[0:64, QC : 2 * QC], z2[:, QC : 2 * QC],
                            mybir.AluOpType.mult,
                        )

                    # (3,2)'s normalize gates the drain's p3 partials: keep
                    # it on ACT (low latency); the DVE chain's ~7us latency
                    # only suits groups deep inside the qc3 exp stream
                    use_dve = (qc, hp) in ((2, 3), (3, 0), (3, 1))
                    deferred.append(normalize_dve if use_dve else normalize_act)
                    if hp == HP - 1 and qc < NQC - 1:
                        # this q-chunk's output projection becomes filler --
                        # but hold it back until qc2 is done, so the final
                        # (filler-starved) q-chunk has PE work to hide its
                        # exp latencies behind
                        for qb in range(4):
                            for ec in range(E // QC):
                                fillers.append(
                                    (f"p3_{qc}_{qb}_{ec}", unit_p3(qc, qb, ec))
                                )

            # drain: flush the last normalize (ACT+DVE), overlap it with the
            # remaining p3 partial chains on the PE, then the f3+store finals
            while deferred:
                deferred.pop()()
            for u in range(2, 8):
                for _cost, fn in unit_p3_last_partial(u):
                    fn()
            emit_steps_until(lambda: False)
            for u in range(8):
                p3_last_final(u)
    return nc


def _make_mask():
    import ml_dtypes

    kk = np.arange(128)[:, None]
    cc = np.arange(128)[None, :]
    return (kk <= cc).astype(np.float32).astype(ml_dtypes.bfloat16)


def make_in_maps(x, W_q, W_k, W_v, W_o):
    import ml_dtypes

    bf16 = ml_dtypes.bfloat16
    mask = _make_mask()
    in_maps = []
    def repack_qk(w):
        # [E, GF] -> [hp, pi, po*128]: contiguous per partition line
        return np.ascontiguousarray(
            w.reshape(NE, 128, HP, 128).transpose(2, 1, 0, 3).reshape(HP, 128, -1)
        )

    def repack_o(w):
        # [GF, E] -> [pi, po*E]: contiguous per partition line.  Negated:
        # the kernel stages hf as -hidden/rowsum (the DVE reciprocal chain
        # converges to -1/rowsum), so (-hf) @ (-W_o) restores the sign.
        return np.ascontiguousarray(
            -w.reshape(HP, 128, E).transpose(1, 0, 2).reshape(128, -1)
        )

    def repack_x(xb):
        # [C, E] -> [NQC, 128, NE, QC]: per q-chunk, contiguous per
        # partition line (one descriptor-light dma_start per chunk)
        return np.ascontiguousarray(
            xb.reshape(NQC, QC, NE, 128).transpose(0, 3, 2, 1)
        )

    def repack_v(w):
        # [E, GF] -> [128, NE, GF]: contiguous per partition line
        return np.ascontiguousarray(w.reshape(NE, 128, GF).transpose(1, 0, 2))

    for i in range(N_CORES):
        b, g = i // 2, i % 2
        in_maps.append(
            {
                "xT": repack_x(np.asarray(x)[b]).astype(bf16),
                "Wq": repack_qk(np.asarray(W_q)[:, g * GF : (g + 1) * GF]).astype(bf16),
                "Wk": repack_qk(np.asarray(W_k)[:, g * GF : (g + 1) * GF]).astype(bf16),
                "Wv": repack_v(np.asarray(W_v)[:, g * GF : (g + 1) * GF]).astype(bf16),
                "Wo": repack_o(np.asarray(W_o)[g * GF : (g + 1) * GF, :]).astype(bf16),
                "mask": mask,
            }
        )
    return in_maps


def kernel(x, W_q, W_k, W_v, W_o):
    global _CACHED_NC
    from concourse.bass_utils import run_bass_kernel_spmd

    if _CACHED_NC is None:
        _CACHED_NC = build_nc()
    nc = _CACHED_NC

    in_maps = make_in_maps(x, W_q, W_k, W_v, W_o)
    res = run_bass_kernel_spmd(nc, in_maps, core_ids=list(range(N_CORES)))
    out = np.empty((B, C, E), dtype=np.float32)
    for b in range(B):
        out[b] = np.asarray(res.results[2 * b]["out"], dtype=np.float32) + np.asarray(
            res.results[2 * b + 1]["out"], dtype=np.float32
        )
    return out

